# revision 18
# baseline (speedup 1.0000x reference)
"""LocalAttention (B=1, S=4096, D=1024, H=16, hd=64, window=128) on 8 trn2 cores.

Sequence-parallel sharding: core c owns queries [512c, 512c+512) plus a K/V
halo of 768 rows ([512c-128, 512c+640), zero-padded at global edges). All
projection weights replicated (bf16); everything on device is bf16 with fp32
PSUM accumulation.

Schedule (single PE stream; ACT/DVE/GpSimd trail one stage behind):
  warmup dummy matmuls (HAM clock) while the first DMAs land
  vproj (eh-outer, kb-inner, DMA arrival order)
  qproj (eb 0..7)
  for i in 0..9:   # hA,hB = heads of eb i-1;  hC,hD = heads of eb i-2
      kproj(i,h1); scores(hA); kproj(i,h2); pv(hC); scores(hB); pv(hD);
      transpose(eb i-2); oproj half1 (eb 0..3) for qb i-5
  tail: oproj half2 (eb 4..7) + combine, per qb

DMA: ~15 large batched descriptors issued in consumption order (descriptor
issue is serial on the sync queue, ~0.6-2us each — keep the count low).

Score tile per (h, qb) is [128 kt, 384] with column order [diag0|diag2|mid]
so one combined [128,256] mask multiply covers both triangles; mid needs no
mask. Edge cores get zeroed triangles via per-core mask DATA (msk[qb]).
Mask multiplies alternate between DVE and GpSimd (SBUF-only op) and the
transpose copies alternate between ACT and DVE to balance the trailing
engines. PV accumulates per-head into one PSUM bank [128, 4*65]; col 64 of
each group is the softmax denominator (ones-column of v). One reciprocal
[128,4] + one broadcast tensor_tensor normalizes the whole head.
"""

import os

import numpy as np
import ml_dtypes

import concourse.bass as bass
import concourse.bacc as bacc
import concourse.mybir as mybir
import concourse.tile as tile
from concourse.bass_utils import run_bass_kernel_spmd

BF16 = mybir.dt.bfloat16
FP32 = mybir.dt.float32

NCORES = 8
S = 4096
D = 1024
H = 16
HD = 64
E = H * HD  # 1024
WIN = 128
SL = S // NCORES       # 512 queries per core
SK = SL + 2 * WIN      # 768 keys/values incl. halo
NQB = SL // 128        # 4 query blocks
NKB = SK // 128        # 6 key blocks
NDB = D // 128         # 8 contraction blocks
NEB = E // 128         # 8 embed blocks
VROW = HD + 1          # 65: v columns per head incl. ones column

_CACHE = {}
LAST_RESULT = None  # BassKernelResults of the most recent run (for test.py)


def _build_nc():
    nc = bacc.Bacc("TRN2", target_bir_lowering=False, debug=False)

    qt_d = nc.dram_tensor("qt", [D, SL], BF16, kind="ExternalInput").ap()
    kt_d = nc.dram_tensor("kt", [D, SK], BF16, kind="ExternalInput").ap()
    vt_d = nc.dram_tensor("vt", [D, SK], BF16, kind="ExternalInput").ap()
    wq_d = nc.dram_tensor("wq", [D, E], BF16, kind="ExternalInput").ap()
    wk_d = nc.dram_tensor("wk", [D, E], BF16, kind="ExternalInput").ap()
    wv_d = nc.dram_tensor("wv", [D, E], BF16, kind="ExternalInput").ap()
    wo_d = nc.dram_tensor("wo", [E, D], BF16, kind="ExternalInput").ap()
    msk_d = nc.dram_tensor("msk", [NQB, 128, 256], BF16, kind="ExternalInput").ap()
    idn_d = nc.dram_tensor("idn", [128, 128], BF16, kind="ExternalInput").ap()
    out_d = nc.dram_tensor("out", [SL, D], BF16, kind="ExternalOutput").ap()

    scale = 1.0 / np.sqrt(HD)

    with tile.TileContext(nc) as tc:
        pools = []

        def pool(name, bufs, **kw):
            p = tc.tile_pool(name=name, bufs=bufs, **kw)
            pools.append(p)
            return p.__enter__()

        const = pool("const", 1)
        pp = pool("pp", 3, space="PSUM")       # projection / transpose / oproj
        ps = pool("ps", 4, space="PSUM")       # score tiles
        ppv = pool("ppv", 1, space="PSUM")     # per-head pv accumulator
        ep = pool("expp", 4)                   # per-head exp tiles (sbuf)
        rp = pool("recip", 4)

        # ---- persistent SBUF tensors ----
        wq_sb = const.tile([128, NDB * E], BF16, tag="wq")
        wk_sb = const.tile([128, NDB * E], BF16, tag="wk")
        wv_sb = const.tile([128, NDB * E], BF16, tag="wv")
        wo_sb = const.tile([128, NEB * D], BF16, tag="wo")
        qtin_sb = const.tile([128, NDB * SL], BF16, tag="qtin")
        ktin_sb = const.tile([128, NDB * SK], BF16, tag="ktin")
        vtin_sb = const.tile([128, NDB * SK], BF16, tag="vtin")
        qt_sb = const.tile([128, NEB * SL], BF16, tag="qt")    # [e,s] per e-blk
        kt_sb = const.tile([128, NEB * SK], BF16, tag="kt")
        v_sb = const.tile([128, NKB * H * VROW], BF16, tag="v")  # [s, h*65]/k-blk
        msk_sb = const.tile([128, NQB * 256], BF16, tag="msk")
        idn_sb = const.tile([128, 128], BF16, tag="idn")
        ao_sb = const.tile([128, NQB * E], BF16, tag="ao")     # attn out [sq, e]
        aot_sb = const.tile([128, NEB * SL], BF16, tag="aot")  # transposed [e, sq]
        o1_sb = const.tile([128, NQB * D], BF16, tag="o1")     # oproj eb0-3 part
        o_sb = const.tile([128, NQB * D], BF16, tag="o")

        sync = nc.sync

        # ---- input DMAs: few large descriptors, consumption order ----
        def load3(sb, dr, ncols, db0, db1, c0, c1):
            sync.dma_start(
                sb[:].rearrange("p (b e) -> p b e", e=ncols)[:, db0:db1, c0:c1],
                dr.rearrange("(b p) e -> p b e", p=128)[:, db0:db1, c0:c1],
            )

        load3(vtin_sb, vt_d, SK, 0, 4, 0, 384)
        load3(wv_sb, wv_d, E, 0, 4, 0, 512)
        load3(vtin_sb, vt_d, SK, 4, 8, 0, 384)
        load3(wv_sb, wv_d, E, 4, 8, 0, 512)
        load3(vtin_sb, vt_d, SK, 0, 8, 384, SK)
        load3(wv_sb, wv_d, E, 0, 8, 512, E)
        load3(qtin_sb, qt_d, SL, 0, 8, 0, SL)
        load3(wq_sb, wq_d, E, 0, 8, 0, 512)
        load3(wq_sb, wq_d, E, 0, 8, 512, E)
        sync.dma_start(
            msk_sb[:].rearrange("p (m c) -> p m c", c=256),
            msk_d.rearrange("m p c -> p m c"),
        )
        sync.dma_start(idn_sb[:], idn_d[:])
        load3(ktin_sb, kt_d, SK, 0, 8, 0, SK)
        load3(wk_sb, wk_d, E, 0, 8, 0, 512)
        load3(wk_sb, wk_d, E, 0, 8, 512, E)
        load3(wo_sb, wo_d, D, 0, 8, 0, D)

        # ones columns of v_sb (col hd=64 of each head group)
        v3 = v_sb[:].rearrange("p (k h c) -> p k h c", k=NKB, h=H)
        nc.gpsimd.memset(v3[:, :, :, HD:VROW], 1.0)

        # PE clock warmup: the HAM throttle runs the PE at 1.2 GHz until it
        # sees ~3.4us of sustained activity. Burn dummy matmuls while the
        # first input DMAs are still in flight so vproj starts at 2.4 GHz.
        dwm = const.tile([128, 512], BF16, tag="dwm")
        nc.vector.memset(dwm[:], 0.0)
        for _ in range(28):
            pd = pp.tile([128, 512], FP32, tag="ps")
            nc.tensor.matmul(
                pd[:], lhsT=dwm[:, 0:128], rhs=dwm[:], start=True, stop=True
            )

        # ---- v projection (natural): [s, e] = VT[d,s].T @ Wv[d,e] ----
        # eh-outer, kb-inner matches the DMA arrival order above.
        for eh in range(2):
            for kb in range(NKB):
                psv = pp.tile([128, 512], FP32, tag="ps")
                for db in range(NDB):
                    nc.tensor.matmul(
                        psv[:],
                        lhsT=vtin_sb[:, db * SK + kb * 128: db * SK + (kb + 1) * 128],
                        rhs=wv_sb[:, db * E + eh * 512: db * E + (eh + 1) * 512],
                        start=(db == 0),
                        stop=(db == NDB - 1),
                    )
                dst = v3[:, kb, eh * 8:(eh + 1) * 8, 0:HD]
                src = psv[:].rearrange("p (h c) -> p h c", c=HD)
                nc.scalar.copy(dst, src)

        # ---- q projection: [e, s] = Wq[d,e].T @ QT[d,s] ----
        for eb in range(NEB):
            psq = pp.tile([128, 512], FP32, tag="ps")
            for db in range(NDB):
                nc.tensor.matmul(
                    psq[:],
                    lhsT=wq_sb[:, db * E + eb * 128: db * E + (eb + 1) * 128],
                    rhs=qtin_sb[:, db * SL: db * SL + SL],
                    start=(db == 0),
                    stop=(db == NDB - 1),
                )
            nc.vector.tensor_copy(qt_sb[:, eb * SL:(eb + 1) * SL], psq[:])

        # ---- pipelined kproj / scores / pv / transpose / oproj-half1 ----
        def kproj(eb, half):
            s0, s1 = (0, 384) if half == 0 else (384, SK)
            psk = pp.tile([128, 512], FP32, tag="ps")
            for db in range(NDB):
                nc.tensor.matmul(
                    psk[:, : s1 - s0],
                    lhsT=wk_sb[:, db * E + eb * 128: db * E + (eb + 1) * 128],
                    rhs=ktin_sb[:, db * SK + s0: db * SK + s1],
                    start=(db == 0),
                    stop=(db == NDB - 1),
                )
            nc.vector.tensor_copy(
                kt_sb[:, eb * SK + s0: eb * SK + s1], psk[:, : s1 - s0]
            )

        # score column order within a [128, 384] tile: [diag0 | diag2 | mid]
        # diag0 -> kb = qb, diag2 -> kb = qb + 2, mid -> kb = qb + 1
        KOFF = (0, 2, 1)

        def scores(h, expp, qbs):
            hp = (h % 2) * HD
            he = h // 2
            qh = qt_sb[hp:hp + HD]
            kh = kt_sb[hp:hp + HD]
            for qb in qbs:
                pscr = ps.tile([128, 384], FP32, tag="scr")
                for r in range(3):
                    kb = qb + KOFF[r]
                    nc.tensor.matmul(
                        pscr[:, r * 128:(r + 1) * 128],
                        lhsT=kh[:, he * SK + kb * 128: he * SK + (kb + 1) * 128],
                        rhs=qh[:, he * SL + qb * 128: he * SL + (qb + 1) * 128],
                        start=True,
                        stop=True,
                    )
                nc.scalar.activation(
                    expp[:, qb * 384:(qb + 1) * 384], pscr[:],
                    mybir.ActivationFunctionType.Exp, scale=scale,
                )
                # one combined triangle mask for [diag0 | diag2]
                nc.vector.tensor_mul(
                    expp[:, qb * 384: qb * 384 + 256],
                    expp[:, qb * 384: qb * 384 + 256],
                    msk_sb[:, qb * 256:(qb + 1) * 256],
                )

        def pv(h, expp):
            pvh = ppv.tile([128, NQB * VROW], FP32, tag="pv")
            for qb in range(NQB):
                # mid slice (unmasked) first: starts accumulation earliest
                for j, r in enumerate((2, 0, 1)):
                    kb = qb + KOFF[r]
                    nc.tensor.matmul(
                        pvh[:, qb * VROW:(qb + 1) * VROW],
                        lhsT=expp[:, qb * 384 + r * 128: qb * 384 + (r + 1) * 128],
                        rhs=v_sb[:, (kb * H + h) * VROW:(kb * H + h + 1) * VROW],
                        start=(j == 0),
                        stop=(j == 2),
                    )
            pv4 = pvh[:].rearrange("p (q c) -> p q c", c=VROW)
            rd = rp.tile([128, NQB, 1], FP32, tag="rd")
            nc.vector.reciprocal(rd[:], pv4[:, :, HD:VROW])
            # ao[:, qb*E + h*64 : +64] = pv[:, qb, 0:64] * rd[:, qb]  (all qb)
            ao4 = ao_sb[:].rearrange("p (q e) -> p q e", e=E)[:, :, h * HD:(h + 1) * HD]
            rdb = rd[:].broadcast_to([128, NQB, HD])
            nc.vector.tensor_mul(ao4, pv4[:, :, 0:HD], rdb)

        def transpose_eb(eb):
            for qb in range(NQB):
                pt = pp.tile([128, 128], BF16, tag="ps")
                nc.tensor.transpose(
                    pt[:], ao_sb[:, qb * E + eb * 128: qb * E + (eb + 1) * 128],
                    idn_sb[:],
                )
                nc.vector.tensor_copy(
                    aot_sb[:, eb * SL + qb * 128: eb * SL + (qb + 1) * 128], pt[:]
                )

        def oproj_half1(qb):
            # partial output projection over eb 0..3, staged to o1_sb (bf16)
            for dh in range(2):
                pso = pp.tile([128, 512], FP32, tag="ps")
                for eb in range(4):
                    nc.tensor.matmul(
                        pso[:],
                        lhsT=aot_sb[:, eb * SL + qb * 128: eb * SL + (qb + 1) * 128],
                        rhs=wo_sb[:, eb * D + dh * 512: eb * D + (dh + 1) * 512],
                        start=(eb == 0),
                        stop=(eb == 3),
                    )
                nc.vector.tensor_copy(
                    o1_sb[:, qb * D + dh * 512: qb * D + (dh + 1) * 512], pso[:]
                )

        expps = {}

        def new_expp(h):
            expps[h] = ep.tile([128, NQB * 384], BF16, tag="expp", name=f"expp{h}")
            return expps[h]

        for i in range(11):
            hA, hB = 2 * (i - 1), 2 * (i - 1) + 1
            hC, hD = 2 * (i - 2), 2 * (i - 2) + 1
            if i < 8:
                kproj(i, 0)
            if 1 <= i <= 8:
                scores(hA, new_expp(hA), (0, 1))
            if i < 8:
                kproj(i, 1)
            if 1 <= i <= 8:
                scores(hA, expps[hA], (2, 3))
            if 2 <= i <= 9:
                pv(hC, expps.pop(hC))
            if 1 <= i <= 8:
                scores(hB, new_expp(hB), (0, 1))
            if 2 <= i <= 9:
                pv(hD, expps.pop(hD))
            if 1 <= i <= 8:
                scores(hB, expps[hB], (2, 3))
            # transposes lag pv by one extra iteration so they never wait on
            # the normalize TT that was just enqueued on the DVE
            if 3 <= i <= 10:
                transpose_eb(i - 3)
            if 6 <= i <= 9:
                oproj_half1(i - 6)

        # ---- tail: oproj half2 (eb 4..7) + combine with half1 ----
        for qb in range(NQB):
            for dh in range(2):
                pso = pp.tile([128, 512], FP32, tag="ps")
                for eb in range(4, NEB):
                    nc.tensor.matmul(
                        pso[:],
                        lhsT=aot_sb[:, eb * SL + qb * 128: eb * SL + (qb + 1) * 128],
                        rhs=wo_sb[:, eb * D + dh * 512: eb * D + (dh + 1) * 512],
                        start=(eb == 4),
                        stop=(eb == NEB - 1),
                    )
                nc.vector.tensor_tensor(
                    o_sb[:, qb * D + dh * 512: qb * D + (dh + 1) * 512],
                    pso[:],
                    o1_sb[:, qb * D + dh * 512: qb * D + (dh + 1) * 512],
                    mybir.AluOpType.add,
                )
            sync.dma_start(
                out_d[qb * 128:(qb + 1) * 128, :], o_sb[:, qb * D:(qb + 1) * D]
            )

        for p in reversed(pools):
            p.__exit__(None, None, None)

    nc.compile()
    return nc


def _host_inputs(query, key, value, Wq, Wk, Wv, Wo):
    bf = ml_dtypes.bfloat16
    q2 = np.ascontiguousarray(query.reshape(S, D))
    k2 = np.asarray(key).reshape(S, D)
    v2 = np.asarray(value).reshape(S, D)
    kpad = np.zeros((S + 2 * WIN, D), np.float32)
    kpad[WIN:WIN + S] = k2
    vpad = np.zeros((S + 2 * WIN, D), np.float32)
    vpad[WIN:WIN + S] = v2

    wq = np.ascontiguousarray(Wq.astype(bf))
    wk = np.ascontiguousarray(Wk.astype(bf))
    wv = np.ascontiguousarray(Wv.astype(bf))
    wo = np.ascontiguousarray(Wo.astype(bf))
    idn = np.eye(128, dtype=bf)

    kt = np.arange(128)[:, None]
    qi = np.arange(128)[None, :]
    tri0 = (qi <= kt).astype(bf)   # diag0: keep qi <= kt
    tri2 = (kt <= qi).astype(bf)   # diag2: keep kt <= qi
    zeros = np.zeros((128, 128), bf)

    in_maps = []
    for c in range(NCORES):
        s0 = c * SL
        qt = np.ascontiguousarray(q2[s0:s0 + SL].T.astype(bf))
        ktc = np.ascontiguousarray(kpad[s0:s0 + SK].T.astype(bf))
        vtc = np.ascontiguousarray(vpad[s0:s0 + SK].T.astype(bf))
        msk = np.empty((NQB, 128, 256), bf)
        for qb in range(NQB):
            m0 = zeros if (c == 0 and qb == 0) else tri0
            m2 = zeros if (c == NCORES - 1 and qb == NQB - 1) else tri2
            msk[qb, :, 0:128] = m0
            msk[qb, :, 128:256] = m2
        in_maps.append({
            "qt": qt, "kt": ktc, "vt": vtc,
            "wq": wq, "wk": wk, "wv": wv, "wo": wo,
            "msk": msk, "idn": idn,
        })
    return in_maps


def kernel(query, key, value, Wq, Wk, Wv, Wo):
    global LAST_RESULT
    if "nc" not in _CACHE:
        _CACHE["nc"] = _build_nc()
    nc = _CACHE["nc"]
    in_maps = _host_inputs(
        np.asarray(query), np.asarray(key), np.asarray(value),
        np.asarray(Wq), np.asarray(Wk), np.asarray(Wv), np.asarray(Wo),
    )
    trace = os.environ.get("KERNEL_TRACE", "0") == "1"
    try:
        res = run_bass_kernel_spmd(
            nc, in_maps, core_ids=list(range(NCORES)), trace=trace
        )
    except ModuleNotFoundError:
        res = run_bass_kernel_spmd(
            nc, in_maps, core_ids=list(range(NCORES)), trace=False
        )
    LAST_RESULT = res
    out = np.concatenate(
        [np.asarray(res.results[c]["out"]) for c in range(NCORES)], axis=0
    )
    return out.reshape(1, S, D).astype(np.float32)
